# Initial kernel scaffold
#
"""Trainium2 Bass kernel for the AttentionRNNModel problem.

Math (fp32 reference):
    xi  = x @ W_i2h.T + b_i2h                      # [B,T,H]
    h_t = tanh(xi_t + h_{t-1} @ W_h2h.T + b_h2h)   # 512 sequential steps
    out = concat_t(h_t) @ W_fc.T + b_fc            # [B, O]

Strategy: data-parallel over batch across 8 cores (16 rows each). On each
core the hidden state is kept transposed, h_T [H=8x128 partitions, 16], so
each step is 8 m-slabs x 8 k-tiles of W_h2h.T as stationary [128,128] bf16
weights with h_T k-tiles moving (N=16), accumulating in one PSUM bank
[128, 8x16]. The input projection + fused bias (b_i2h + b_h2h) enter the
same PSUM group as one extra K=65 matmul per m-slab (x_t augmented with a
ones row). tanh runs as a single ScalarE activation PSUM->SBUF (bf16),
producing the next step's h_T in-place in the layout the matmuls need —
no transposes anywhere in the loop. The final FC is folded in as 8 more
matmuls per step accumulating W_fc_t.T @ h_t into a tiny [24,16] PSUM
accumulator that stays open across all 512 steps; only [24,16] per core is
DMA'd out at the end. W_fc (bf16, 25MB) streams in per-step; everything
else is SBUF-resident.
"""

import numpy as np
import ml_dtypes

import concourse.bass as bass
import concourse.tile as tile
from concourse import bacc, mybir
from concourse.bass_utils import run_bass_kernel_spmd

B, T, D, H, O = 128, 512, 64, 1024, 24
NCORES = 8
BC = B // NCORES          # batch per core = 16
KM = H // 128             # 8 k-tiles / m-slabs
BF16 = mybir.dt.bfloat16
F32 = mybir.dt.float32
bf16 = ml_dtypes.bfloat16


def _build_program(t_steps: int):
    nc = bacc.Bacc("TRN2", target_bir_lowering=False, debug=False)

    wT_d = nc.dram_tensor("wT", [128, KM, H], BF16, kind="ExternalInput")
    wiT_d = nc.dram_tensor("wiT", [D + 1, KM, 128], BF16, kind="ExternalInput")
    xTa_d = nc.dram_tensor("xTa", [D + 1, t_steps, BC], BF16, kind="ExternalInput")
    wfc_d = nc.dram_tensor("wfc", [t_steps, 128, KM, O], BF16, kind="ExternalInput")
    out_d = nc.dram_tensor("out", [O, BC], F32, kind="ExternalOutput")

    with tile.TileContext(nc) as tc:
        with (
            tc.tile_pool(name="const", bufs=1) as const_pool,
            tc.tile_pool(name="wfc", bufs=4) as wfc_pool,
            tc.tile_pool(name="h", bufs=2) as h_pool,
            tc.tile_pool(name="ps", bufs=2, space=bass.MemorySpace.PSUM) as ps_pool,
            tc.tile_pool(name="fcps", bufs=1, space=bass.MemorySpace.PSUM) as fcps_pool,
            tc.tile_pool(name="outp", bufs=1) as out_pool,
        ):
            wT = const_pool.tile([128, KM, H], BF16)
            nc.sync.dma_start(wT[:], wT_d[:])
            wiT = const_pool.tile([D + 1, KM, 128], BF16)
            nc.sync.dma_start(wiT[:], wiT_d[:])
            xTa = const_pool.tile([D + 1, t_steps, BC], BF16)
            nc.sync.dma_start(xTa[:], xTa_d[:])

            fc_ps = fcps_pool.tile([O, BC], F32)

            h_prev = None
            wfc_prev = None
            for t in range(t_steps):
                wfc_t = wfc_pool.tile([128, KM, O], BF16)
                nc.sync.dma_start(wfc_t[:], wfc_d[t])

                ps = ps_pool.tile([128, KM, BC], F32)
                first = True
                for m in range(KM):
                    if t > 0:
                        for k in range(KM):
                            nc.tensor.matmul(
                                ps[:, m, :],
                                wT[:, k, m * 128:(m + 1) * 128],
                                h_prev[:, k, :],
                                start=first,
                                stop=False,
                            )
                            first = False
                    nc.tensor.matmul(
                        ps[:, m, :],
                        wiT[:, m, :],
                        xTa[:, t, :],
                        start=first,
                        stop=(m == KM - 1),
                    )
                    first = False

                # FC contribution of h_{t-1}: overlaps PE with tanh(t) on ScalarE
                if t > 0:
                    for k in range(KM):
                        nc.tensor.matmul(
                            fc_ps[:],
                            wfc_prev[:, k, :],
                            h_prev[:, k, :],
                            start=(t == 1 and k == 0),
                            stop=False,
                        )

                h_new = h_pool.tile([128, KM, BC], BF16)
                nc.scalar.activation(
                    h_new[:], ps[:], mybir.ActivationFunctionType.Tanh
                )
                h_prev = h_new
                wfc_prev = wfc_t

            for k in range(KM):
                nc.tensor.matmul(
                    fc_ps[:],
                    wfc_prev[:, k, :],
                    h_prev[:, k, :],
                    start=False,
                    stop=(k == KM - 1),
                )

            out_sb = out_pool.tile([O, BC], F32)
            nc.vector.tensor_copy(out_sb[:], fc_ps[:])
            nc.sync.dma_start(out_d[:], out_sb[:])

    nc.compile()
    return nc


def _prep_inputs(x, W_i2h, b_i2h, W_h2h, b_h2h, W_fc, t_steps):
    b_total = (np.asarray(b_i2h) + np.asarray(b_h2h)).astype(np.float32)

    # wT[p, kb, c] = W_h2h[c, kb*128+p]
    wT = np.ascontiguousarray(
        np.asarray(W_h2h).T.reshape(KM, 128, H).transpose(1, 0, 2)
    ).astype(bf16)

    # wiT[p<64, m, j] = W_i2h[m*128+j, p];  wiT[64, m, j] = b_total[m*128+j]
    wiT = np.empty((D + 1, KM, 128), np.float32)
    wiT[:D] = np.asarray(W_i2h).T.reshape(D, KM, 128)
    wiT[D] = b_total.reshape(KM, 128)
    wiT = wiT.astype(bf16)

    # wfc[t, p, k, o] = W_fc[o, t*1024 + k*128 + p]
    wfc = np.ascontiguousarray(
        np.asarray(W_fc).reshape(O, t_steps, KM, 128).transpose(1, 3, 2, 0)
    ).astype(bf16)

    # per-core xTa[p<64, t, b] = x[c*BC+b, t, p]; xTa[64] = 1.0
    xT = np.asarray(x)[:, :t_steps, :].transpose(2, 1, 0)  # [D, T, B]
    xTas = []
    for c in range(NCORES):
        xa = np.empty((D + 1, t_steps, BC), np.float32)
        xa[:D] = xT[:, :, c * BC:(c + 1) * BC]
        xa[D] = 1.0
        xTas.append(xa.astype(bf16))
    return wT, wiT, wfc, xTas


def _run(x, W_i2h, b_i2h, W_h2h, b_h2h, W_fc, b_fc, t_steps=T, trace=False):
    wT, wiT, wfc, xTas = _prep_inputs(x, W_i2h, b_i2h, W_h2h, b_h2h, W_fc, t_steps)
    nc = _build_program(t_steps)
    in_maps = [
        {"wT": wT, "wiT": wiT, "xTa": xTas[c], "wfc": wfc} for c in range(NCORES)
    ]
    res = run_bass_kernel_spmd(
        nc, in_maps, core_ids=list(range(NCORES)), trace=trace,
        **({"trace_cores": list(range(NCORES))} if trace else {}),
    )
    out = np.empty((B, O), np.float32)
    for c in range(NCORES):
        out[c * BC:(c + 1) * BC, :] = res.results[c]["out"].T
    out += np.asarray(b_fc, np.float32)[None, :]
    return out, res


def kernel(x, batchSize, W_i2h, b_i2h, W_h2h, b_h2h, W_fc, b_fc):
    out, _ = _run(x, W_i2h, b_i2h, W_h2h, b_h2h, W_fc, b_fc)
    return out


# revision 4
# speedup vs baseline: 10.9542x; 10.9542x over previous
"""Trainium2 Bass kernel for the AttentionRNNModel problem.

Math (fp32 reference):
    xi  = x @ W_i2h.T + b_i2h                      # [B,T,H]
    h_t = tanh(xi_t + h_{t-1} @ W_h2h.T + b_h2h)   # 512 sequential steps
    out = concat_t(h_t) @ W_fc.T + b_fc            # [B, O]

Strategy: data-parallel over batch across 8 cores (16 rows each). On each
core the hidden state is kept transposed, h_T [H=8x128 partitions, 16], so
each step is 8 m-slabs x 8 k-tiles of W_h2h.T as stationary [128,128] bf16
weights with h_T k-tiles moving (N=16), accumulating in one PSUM bank
[128, 8x16]. The input projection + fused bias (b_i2h + b_h2h) enter the
same PSUM group as one extra K=65 matmul per m-slab (x_t augmented with a
ones row). tanh runs as a single ScalarE activation PSUM->SBUF (bf16),
producing the next step's h_T in-place in the layout the matmuls need —
no transposes anywhere in the loop. The final FC is folded in as 8 more
matmuls per step accumulating W_fc_t.T @ h_t into a tiny [24,16] PSUM
accumulator that stays open across all 512 steps; only [24,16] per core is
DMA'd out at the end. W_fc (bf16, 25MB) streams in per-step; everything
else is SBUF-resident.
"""

import numpy as np
import ml_dtypes

import concourse.bass as bass
import concourse.tile as tile
from concourse import bacc, mybir
from concourse.bass_utils import run_bass_kernel_spmd

B, T, D, H, O = 128, 512, 64, 1024, 24
NCORES = 8
BC = B // NCORES          # batch per core = 16
KM = H // 128             # 8 k-tiles / m-slabs
BF16 = mybir.dt.bfloat16
F32 = mybir.dt.float32
bf16 = ml_dtypes.bfloat16


def _build_program(t_steps: int, reps: int = 1):
    nc = bacc.Bacc("TRN2", target_bir_lowering=False, debug=False)

    wT_d = nc.dram_tensor("wT", [128, KM, H], BF16, kind="ExternalInput")
    wiT_d = nc.dram_tensor("wiT", [D + 1, KM, 128], BF16, kind="ExternalInput")
    xTa_d = nc.dram_tensor("xTa", [D + 1, t_steps, BC], BF16, kind="ExternalInput")
    wfc_d = nc.dram_tensor("wfc", [t_steps, 128, KM, O], BF16, kind="ExternalInput")
    out_d = nc.dram_tensor("out", [O, BC], F32, kind="ExternalOutput")

    with tile.TileContext(nc) as tc:
        with (
            tc.tile_pool(name="const", bufs=1) as const_pool,
            tc.tile_pool(name="wfc", bufs=4) as wfc_pool,
            tc.tile_pool(name="h", bufs=2) as h_pool,
            tc.tile_pool(name="ps", bufs=2, space=bass.MemorySpace.PSUM) as ps_pool,
            tc.tile_pool(name="fcps", bufs=1, space=bass.MemorySpace.PSUM) as fcps_pool,
            tc.tile_pool(name="outp", bufs=1) as out_pool,
        ):
            wT = const_pool.tile([128, KM, H], BF16)
            nc.sync.dma_start(wT[:], wT_d[:])
            wiT = const_pool.tile([D + 1, KM, 128], BF16)
            nc.sync.dma_start(wiT[:], wiT_d[:])
            xTa = const_pool.tile([D + 1, t_steps, BC], BF16)
            nc.sync.dma_start(xTa[:], xTa_d[:])

            fc_ps = fcps_pool.tile([O, BC], F32)

            import contextlib
            rep_ctx = tc.For_i(0, reps) if reps > 1 else contextlib.nullcontext()
            with rep_ctx:
                _emit_body(
                    nc, tc, t_steps, wT, wiT, xTa, fc_ps,
                    wfc_pool, h_pool, ps_pool, out_pool, wfc_d, out_d,
                )

    nc.compile()
    return nc


def _emit_body(nc, tc, t_steps, wT, wiT, xTa, fc_ps,
               wfc_pool, h_pool, ps_pool, out_pool, wfc_d, out_d):
            h_prev = None
            wfc_prev = None
            for t in range(t_steps):
                wfc_t = wfc_pool.tile([128, KM, O], BF16)
                nc.sync.dma_start(wfc_t[:], wfc_d[t])

                ps = ps_pool.tile([128, KM, BC], F32)
                first = True
                for m in range(KM):
                    if t > 0:
                        for k in range(KM):
                            nc.tensor.matmul(
                                ps[:, m, :],
                                wT[:, k, m * 128:(m + 1) * 128],
                                h_prev[:, k, :],
                                start=first,
                                stop=False,
                            )
                            first = False
                    nc.tensor.matmul(
                        ps[:, m, :],
                        wiT[:, m, :],
                        xTa[:, t, :],
                        start=first,
                        stop=(m == KM - 1),
                    )
                    first = False

                # FC contribution of h_{t-1}: overlaps PE with tanh(t) on ScalarE
                if t > 0:
                    for k in range(KM):
                        nc.tensor.matmul(
                            fc_ps[:],
                            wfc_prev[:, k, :],
                            h_prev[:, k, :],
                            start=(t == 1 and k == 0),
                            stop=False,
                        )

                h_new = h_pool.tile([128, KM, BC], BF16)
                nc.scalar.activation(
                    h_new[:], ps[:], mybir.ActivationFunctionType.Tanh
                )
                h_prev = h_new
                wfc_prev = wfc_t

            for k in range(KM):
                nc.tensor.matmul(
                    fc_ps[:],
                    wfc_prev[:, k, :],
                    h_prev[:, k, :],
                    start=False,
                    stop=(k == KM - 1),
                )

            out_sb = out_pool.tile([O, BC], F32)
            nc.vector.tensor_copy(out_sb[:], fc_ps[:])
            nc.sync.dma_start(out_d[:], out_sb[:])


def _prep_inputs(x, W_i2h, b_i2h, W_h2h, b_h2h, W_fc, t_steps):
    b_total = (np.asarray(b_i2h) + np.asarray(b_h2h)).astype(np.float32)

    # wT[p, kb, c] = W_h2h[c, kb*128+p]
    wT = np.ascontiguousarray(
        np.asarray(W_h2h).T.reshape(KM, 128, H).transpose(1, 0, 2)
    ).astype(bf16)

    # wiT[p<64, m, j] = W_i2h[m*128+j, p];  wiT[64, m, j] = b_total[m*128+j]
    wiT = np.empty((D + 1, KM, 128), np.float32)
    wiT[:D] = np.asarray(W_i2h).T.reshape(D, KM, 128)
    wiT[D] = b_total.reshape(KM, 128)
    wiT = wiT.astype(bf16)

    # wfc[t, p, k, o] = W_fc[o, t*1024 + k*128 + p]
    wfc = np.ascontiguousarray(
        np.asarray(W_fc)[:, :t_steps * H]
        .reshape(O, t_steps, KM, 128).transpose(1, 3, 2, 0)
    ).astype(bf16)

    # per-core xTa[p<64, t, b] = x[c*BC+b, t, p]; xTa[64] = 1.0
    xT = np.asarray(x)[:, :t_steps, :].transpose(2, 1, 0)  # [D, T, B]
    xTas = []
    for c in range(NCORES):
        xa = np.empty((D + 1, t_steps, BC), np.float32)
        xa[:D] = xT[:, :, c * BC:(c + 1) * BC]
        xa[D] = 1.0
        xTas.append(xa.astype(bf16))
    return wT, wiT, wfc, xTas


def _run(x, W_i2h, b_i2h, W_h2h, b_h2h, W_fc, b_fc, t_steps=T, trace=False):
    wT, wiT, wfc, xTas = _prep_inputs(x, W_i2h, b_i2h, W_h2h, b_h2h, W_fc, t_steps)
    nc = _build_program(t_steps)
    in_maps = [
        {"wT": wT, "wiT": wiT, "xTa": xTas[c], "wfc": wfc} for c in range(NCORES)
    ]
    res = run_bass_kernel_spmd(
        nc, in_maps, core_ids=list(range(NCORES)), trace=trace,
        **({"trace_cores": list(range(NCORES))} if trace else {}),
    )
    out = np.empty((B, O), np.float32)
    for c in range(NCORES):
        out[c * BC:(c + 1) * BC, :] = res.results[c]["out"].T
    out += np.asarray(b_fc, np.float32)[None, :]
    return out, res


def kernel(x, batchSize, W_i2h, b_i2h, W_h2h, b_h2h, W_fc, b_fc):
    out, _ = _run(x, W_i2h, b_i2h, W_h2h, b_h2h, W_fc, b_fc)
    return out
